# revision 6
# baseline (speedup 1.0000x reference)
"""Multi-head attention (B=2, S=2048, D=1024, H=16, dk=64) on 8 TRN2 NeuronCores.

Sharding: core c handles batch b=c//4 and head group g=c%4 (4 heads each).
Per-core Bass/Tile kernel computes, for its 4 heads:
  qT/kT = (x Wq/Wk + b)^T in [dk, S] layout, V = x Wv in [S, dk] layout (no bias),
  transposed scores S^T = K Q^T on the PE (fp32r), U^T = exp(0.125 * S^T) on ACT,
  row-sums via an appended ones-column on V (PE), normalized attention written
  to HBM transposed per head ([k, q]), attention output re-projected with Wo.
Host side: shard/unshard (transpose of per-head attn shards), sum of the 4
partial output projections per batch, plus the rank-1 bias corrections
(attn rows sum to 1 so V's bias contributes bv @ Wo exactly).
"""

import numpy as np

import concourse.bass as bass
import concourse.tile as tile
import concourse.bass_utils as bass_utils
from concourse import bacc, mybir

F32 = mybir.dt.float32
F32R = mybir.dt.float32r
EXP = mybir.ActivationFunctionType.Exp

S = 2048          # sequence length
D = 1024          # model dim
DK = 64           # head dim
NHC = 4           # heads per core
P = 128           # partitions
SCALE = 0.125     # 1/sqrt(dk)

_CACHE: dict = {}


def _build_module():
    nc = bacc.Bacc(
        "TRN2",
        target_bir_lowering=False,
        debug=False,
        enable_asserts=False,
        num_devices=8,
    )

    xT_d = nc.dram_tensor("xT", [D, S], F32R, kind="ExternalInput").ap()
    wq_d = nc.dram_tensor("wq", [D, 256], F32R, kind="ExternalInput").ap()
    wk_d = nc.dram_tensor("wk", [D, 256], F32R, kind="ExternalInput").ap()
    wv_d = nc.dram_tensor("wv", [D, 256], F32R, kind="ExternalInput").ap()
    wo_d = nc.dram_tensor("wo", [256, D], F32R, kind="ExternalInput").ap()
    bqk_d = nc.dram_tensor("bqk", [P, 4], F32, kind="ExternalInput").ap()
    attnT_d = nc.dram_tensor("attnT", [NHC, S, S], F32R, kind="ExternalOutput").ap()
    outp_d = nc.dram_tensor("outp", [S, D], F32, kind="ExternalOutput").ap()

    with tile.TileContext(nc) as tc:
        _emit(nc, tc, xT_d, wq_d, wk_d, wv_d, wo_d, bqk_d, attnT_d, outp_d)

    nc.compile()
    return nc


def _emit(nc, tc, xT_d, wq_d, wk_d, wv_d, wo_d, bqk_d, attnT_d, outp_d):
    with (
        tc.tile_pool(name="qk", bufs=1) as qk_pool,
        tc.tile_pool(name="v", bufs=1) as v_pool,
        tc.tile_pool(name="ao", bufs=1) as ao_pool,
        tc.tile_pool(name="wo", bufs=1) as wo_pool,
        tc.tile_pool(name="misc", bufs=1) as misc_pool,
        tc.tile_pool(name="rb", bufs=2) as rb_pool,
        tc.tile_pool(name="rinv", bufs=2) as rinv_pool,
    ):
        # persistent tiles
        qt_sb = [qk_pool.tile([P, S], F32R, tag=f"qt{m}", name=f"qt{m}") for m in range(2)]
        kt_sb = [qk_pool.tile([P, S], F32R, tag=f"kt{m}", name=f"kt{m}") for m in range(2)]
        v_sb = [v_pool.tile([P, NHC * 65], F32R, tag=f"v{i}", name=f"v{i}") for i in range(16)]
        ao_sb = [ao_pool.tile([P, S], F32R, tag=f"ao{m}", name=f"ao{m}") for m in range(2)]
        wo_sb = [wo_pool.tile([P, D], F32R, tag=f"wo{t}", name=f"wo{t}") for t in range(2)]
        bias_sb = misc_pool.tile([P, 4], F32, tag="bias", name="bias")

        # ---------------- Phase 1: projections ----------------
        with (
            tc.tile_pool(name="xw", bufs=1) as xw_pool,
            tc.tile_pool(name="psP", bufs=4, space="PSUM") as psP_pool,
            tc.tile_pool(name="psV", bufs=2, space="PSUM") as psV_pool,
        ):
            xt_sb = [xw_pool.tile([P, S], F32R, tag=f"x{d}", name=f"x{d}") for d in range(8)]
            wq_sb = xw_pool.tile([P, 8 * 256], F32R, tag="wq", name="wqs")
            wk_sb = xw_pool.tile([P, 8 * 256], F32R, tag="wk", name="wks")
            wv_sb = xw_pool.tile([P, 8 * 256], F32R, tag="wv", name="wvs")

            nc.sync.dma_start(bias_sb[:], bqk_d)
            for d in range(8):
                nc.sync.dma_start(xt_sb[d][:], xT_d[d * P:(d + 1) * P, :])
            for w_sb, w_d in ((wq_sb, wq_d), (wk_sb, wk_d), (wv_sb, wv_d)):
                nc.sync.dma_start(
                    w_sb[:].rearrange("p (d c) -> p d c", c=256),
                    w_d.rearrange("(d p) c -> p d c", p=P),
                )
            for t in range(2):
                nc.sync.dma_start(wo_sb[t][:], wo_d[t * P:(t + 1) * P, :])

            # qT / kT projections: [dk*2heads, S] per m-group, transposed layout
            for w_sb, dst, bcol in ((wq_sb, qt_sb, 0), (wk_sb, kt_sb, 2)):
                for m in range(2):
                    for sc in range(4):
                        ps = psP_pool.tile([P, 512], F32, tag="p", name="psp")
                        for d in range(8):
                            nc.tensor.matmul(
                                ps[:],
                                lhsT=w_sb[:, d * 256 + m * P: d * 256 + (m + 1) * P],
                                rhs=xt_sb[d][:, sc * 512:(sc + 1) * 512],
                                start=(d == 0),
                                stop=(d == 7),
                            )
                        nc.scalar.add(
                            dst[m][:, sc * 512:(sc + 1) * 512],
                            ps[:],
                            bias_sb[:, bcol + m: bcol + m + 1],
                        )

            # V projection: natural [S, dk] layout, per head padded with a ones
            # column (row-sum trick); no bias (handled on host via bv @ Wo).
            for sc in range(16):
                ps = psV_pool.tile([P, 256], F32, tag="pv", name="psv")
                for d in range(8):
                    nc.tensor.matmul(
                        ps[:],
                        lhsT=xt_sb[d][:, sc * P:(sc + 1) * P],
                        rhs=wv_sb[:, d * 256:(d + 1) * 256],
                        start=(d == 0),
                        stop=(d == 7),
                    )
                nc.gpsimd.memset(v_sb[sc][:].bitcast(F32), 1.0)
                nc.scalar.copy(
                    v_sb[sc][:].rearrange("p (h c) -> p h c", c=65)[:, :, 0:64],
                    ps[:].rearrange("p (h c) -> p h c", c=64),
                )

        # ---------------- Phase 2: attention ----------------
        with (
            tc.tile_pool(name="u", bufs=2) as u_pool,
            tc.tile_pool(name="psS", bufs=2, space="PSUM") as psS_pool,
            tc.tile_pool(name="psO", bufs=2, space="PSUM") as psO_pool,
        ):
            for h in range(NHC):
                m, p0 = h // 2, (h % 2) * DK
                for qs in range(4):
                    qw = slice(qs * 512, (qs + 1) * 512)
                    u_t = u_pool.tile([P, 16 * 512], F32R, tag="u", name="u_t")
                    psO = psO_pool.tile([P, 512], F32, tag="o", name="pso")
                    for kcp in range(8):
                        psS = psS_pool.tile([P, 1024], F32, tag="s", name="pss")
                        for half in range(2):
                            kc = 2 * kcp + half
                            nc.tensor.matmul(
                                psS[:, half * 512:(half + 1) * 512],
                                lhsT=kt_sb[m][p0:p0 + DK, kc * P:(kc + 1) * P],
                                rhs=qt_sb[m][p0:p0 + DK, qw],
                                start=True,
                                stop=True,
                            )
                        nc.scalar.activation(
                            u_t[:, kcp * 1024:(kcp + 1) * 1024], psS[:], EXP, scale=SCALE
                        )
                        for half in range(2):
                            kc = 2 * kcp + half
                            nc.tensor.matmul(
                                psO[0:65, :],
                                lhsT=v_sb[kc][:, h * 65:(h + 1) * 65],
                                rhs=u_t[:, kc * 512:(kc + 1) * 512],
                                start=(kc == 0),
                                stop=(kc == 15),
                            )
                    rinv_t = rinv_pool.tile([1, 512], F32, tag="ri", name="rinv")
                    nc.vector.reciprocal(rinv_t[:], psO[64:65, :])
                    rb_t = rb_pool.tile([P, 512], F32, tag="rb", name="rb")
                    nc.gpsimd.partition_broadcast(rb_t[:], rinv_t[:])
                    # normalize U in-place; 256-wide ops stay under the DVE
                    # drain threshold (~266 ns)
                    for kc in range(16):
                        for hf in range(2):
                            cw = slice(kc * 512 + hf * 256, kc * 512 + (hf + 1) * 256)
                            nc.vector.tensor_mul(
                                u_t[:, cw], u_t[:, cw], rb_t[:, hf * 256:(hf + 1) * 256]
                            )
                    nc.vector.tensor_mul(
                        ao_sb[m][p0:p0 + DK, qw], psO[0:DK, :], rb_t[0:DK, :]
                    )
                    nc.sync.dma_start(
                        attnT_d[h].rearrange("(kc p) q -> p kc q", p=P)[:, :, qw],
                        u_t[:].rearrange("p (kc q) -> p kc q", q=512),
                    )

        # ---------------- Phase 3: output projection ----------------
        with (
            tc.tile_pool(name="psF", bufs=2, space="PSUM") as psF_pool,
            tc.tile_pool(name="oev", bufs=2) as oev_pool,
        ):
            for sc in range(16):
                psF = psF_pool.tile([P, D], F32, tag="f", name="psf")
                for n2 in range(2):
                    for t in range(2):
                        nc.tensor.matmul(
                            psF[:, n2 * 512:(n2 + 1) * 512],
                            lhsT=ao_sb[t][:, sc * P:(sc + 1) * P],
                            rhs=wo_sb[t][:, n2 * 512:(n2 + 1) * 512],
                            start=(t == 0),
                            stop=(t == 1),
                        )
                ot = oev_pool.tile([P, D], F32, tag="oe", name="oev")
                nc.scalar.copy(ot[:], psF[:])
                nc.sync.dma_start(outp_d[sc * P:(sc + 1) * P, :], ot[:])


def get_module():
    if "nc" not in _CACHE:
        _CACHE["nc"] = _build_module()
    return _CACHE["nc"]


def make_in_maps(x, Wq, bq, Wk, bk, Wv, Wo):
    """Shard the full inputs into the 8 per-core input maps."""
    x = np.ascontiguousarray(np.asarray(x, dtype=np.float32))
    xT = [np.ascontiguousarray(x[b].T) for b in range(x.shape[0])]
    in_maps = []
    for c in range(8):
        b, g = c // 4, c % 4
        cs = slice(g * 256, (g + 1) * 256)
        bq_g = np.asarray(bq)[cs]
        bk_g = np.asarray(bk)[cs]
        bqk = np.stack(
            [bq_g[0:128], bq_g[128:256], bk_g[0:128], bk_g[128:256]], axis=1
        ).astype(np.float32)
        in_maps.append(
            {
                "xT": xT[b],
                "wq": np.ascontiguousarray(np.asarray(Wq)[:, cs], dtype=np.float32),
                "wk": np.ascontiguousarray(np.asarray(Wk)[:, cs], dtype=np.float32),
                "wv": np.ascontiguousarray(np.asarray(Wv)[:, cs], dtype=np.float32),
                "wo": np.ascontiguousarray(np.asarray(Wo)[cs, :], dtype=np.float32),
                "bqk": np.ascontiguousarray(bqk),
            }
        )
    return in_maps


def assemble(results, Wv_bias_corr, B=2):
    """Gather the 8 per-core result dicts into full (out, attn_weights)."""
    out = np.empty((B, S, D), np.float32)
    attn = np.empty((B, 16, S, S), np.float32)
    for b in range(B):
        acc = None
        for g in range(4):
            r = results[b * 4 + g]
            acc = r["outp"].copy() if acc is None else acc + r["outp"]
            at = r["attnT"]
            for j in range(4):
                attn[b, 4 * g + j] = at[j].T
        out[b] = acc + Wv_bias_corr[None, :]
    return out, attn


def kernel(x, Wq, bq, Wk, bk, Wv, bv, Wo, bo):
    nc = get_module()
    in_maps = make_in_maps(x, Wq, bq, Wk, bk, Wv, Wo)
    res = bass_utils.run_bass_kernel_spmd(nc, in_maps, core_ids=list(range(8)))
    corr = (np.asarray(bv, np.float32) @ np.asarray(Wo, np.float32)
            + np.asarray(bo, np.float32))
    return assemble(res.results, corr)


# revision 7
# speedup vs baseline: 52436.8554x; 52436.8554x over previous
"""Multi-head attention (B=2, S=2048, D=1024, H=16, dk=64) on 8 TRN2 NeuronCores.

Sharding: core c handles batch b=c//4 and head group g=c%4 (4 heads each).
Per-core Bass/Tile kernel computes, for its 4 heads:
  qT/kT = (x Wq/Wk + b)^T in [dk, S] layout, V = x Wv in [S, dk] layout (no bias),
  transposed scores S^T = K Q^T on the PE (fp32r), U^T = exp(0.125 * S^T) on ACT,
  row-sums via an appended ones-column on V (PE), normalized attention written
  to HBM transposed per head ([k, q]), attention output re-projected with Wo.
Host side: shard/unshard (transpose of per-head attn shards), sum of the 4
partial output projections per batch, plus the rank-1 bias corrections
(attn rows sum to 1 so V's bias contributes bv @ Wo exactly).
"""

import numpy as np

import concourse.bass as bass
import concourse.tile as tile
import concourse.bass_utils as bass_utils
from concourse import bacc, mybir

F32 = mybir.dt.float32
F32R = mybir.dt.float32r
EXP = mybir.ActivationFunctionType.Exp

S = 2048          # sequence length
D = 1024          # model dim
DK = 64           # head dim
NHC = 4           # heads per core
P = 128           # partitions
SCALE = 0.125     # 1/sqrt(dk)

_CACHE: dict = {}
_ABLATE: set = set()   # dev-only: skip parts to attribute time in TimelineSim


def _build_module():
    nc = bacc.Bacc(
        "TRN2",
        target_bir_lowering=False,
        debug=False,
        enable_asserts=False,
        num_devices=8,
    )

    xT_d = nc.dram_tensor("xT", [D, S], F32R, kind="ExternalInput").ap()
    wq_d = nc.dram_tensor("wq", [D, 256], F32R, kind="ExternalInput").ap()
    wk_d = nc.dram_tensor("wk", [D, 256], F32R, kind="ExternalInput").ap()
    wv_d = nc.dram_tensor("wv", [D, 256], F32R, kind="ExternalInput").ap()
    wo_d = nc.dram_tensor("wo", [256, D], F32R, kind="ExternalInput").ap()
    bqk_d = nc.dram_tensor("bqk", [P, 4], F32, kind="ExternalInput").ap()
    attnT_d = nc.dram_tensor("attnT", [NHC, S, S], F32R, kind="ExternalOutput").ap()
    outp_d = nc.dram_tensor("outp", [S, D], F32, kind="ExternalOutput").ap()

    with tile.TileContext(nc) as tc:
        _emit(nc, tc, xT_d, wq_d, wk_d, wv_d, wo_d, bqk_d, attnT_d, outp_d)

    nc.compile()
    return nc


def _emit(nc, tc, xT_d, wq_d, wk_d, wv_d, wo_d, bqk_d, attnT_d, outp_d):
    with (
        tc.tile_pool(name="qk", bufs=1) as qk_pool,
        tc.tile_pool(name="v", bufs=1) as v_pool,
        tc.tile_pool(name="ao", bufs=1) as ao_pool,
        tc.tile_pool(name="wo", bufs=1) as wo_pool,
        tc.tile_pool(name="misc", bufs=1) as misc_pool,
        tc.tile_pool(name="rb", bufs=2) as rb_pool,
        tc.tile_pool(name="rinv", bufs=2) as rinv_pool,
    ):
        # persistent tiles
        qt_sb = [qk_pool.tile([P, S], F32R, tag=f"qt{m}", name=f"qt{m}") for m in range(2)]
        kt_sb = [qk_pool.tile([P, S], F32R, tag=f"kt{m}", name=f"kt{m}") for m in range(2)]
        v_sb = [v_pool.tile([P, NHC * 65], F32R, tag=f"v{i}", name=f"v{i}") for i in range(16)]
        ao_sb = [ao_pool.tile([P, S], F32R, tag=f"ao{m}", name=f"ao{m}") for m in range(2)]
        wo_sb = [wo_pool.tile([P, D], F32R, tag=f"wo{t}", name=f"wo{t}") for t in range(2)]
        bias_sb = misc_pool.tile([P, 4], F32, tag="bias", name="bias")

        # ---------------- Phase 1: projections ----------------
        with (
            tc.tile_pool(name="xw", bufs=1) as xw_pool,
            tc.tile_pool(name="psP", bufs=4, space="PSUM") as psP_pool,
            tc.tile_pool(name="psV", bufs=2, space="PSUM") as psV_pool,
        ):
            xt_sb = [xw_pool.tile([P, S], F32R, tag=f"x{d}", name=f"x{d}") for d in range(8)]
            wq_sb = xw_pool.tile([P, 8 * 256], F32R, tag="wq", name="wqs")
            wk_sb = xw_pool.tile([P, 8 * 256], F32R, tag="wk", name="wks")
            wv_sb = xw_pool.tile([P, 8 * 256], F32R, tag="wv", name="wvs")

            nc.sync.dma_start(bias_sb[:], bqk_d)
            for d in range(8):
                nc.sync.dma_start(xt_sb[d][:], xT_d[d * P:(d + 1) * P, :])
            for w_sb, w_d in ((wq_sb, wq_d), (wk_sb, wk_d), (wv_sb, wv_d)):
                nc.sync.dma_start(
                    w_sb[:].rearrange("p (d c) -> p d c", c=256),
                    w_d.rearrange("(d p) c -> p d c", p=P),
                )
            for t in range(2):
                nc.sync.dma_start(wo_sb[t][:], wo_d[t * P:(t + 1) * P, :])

            # qT / kT projections: [dk*2heads, S] per m-group, transposed layout
            for w_sb, dst, bcol in ((wq_sb, qt_sb, 0), (wk_sb, kt_sb, 2)):
                for m in range(2):
                    for sc in range(4):
                        ps = psP_pool.tile([P, 512], F32, tag="p", name="psp")
                        for d in range(8):
                            nc.tensor.matmul(
                                ps[:],
                                lhsT=w_sb[:, d * 256 + m * P: d * 256 + (m + 1) * P],
                                rhs=xt_sb[d][:, sc * 512:(sc + 1) * 512],
                                start=(d == 0),
                                stop=(d == 7),
                            )
                        nc.scalar.add(
                            dst[m][:, sc * 512:(sc + 1) * 512],
                            ps[:],
                            bias_sb[:, bcol + m: bcol + m + 1],
                        )

            # V projection: natural [S, dk] layout, per head padded with a ones
            # column (row-sum trick); no bias (handled on host via bv @ Wo).
            for sc in range(16):
                ps = psV_pool.tile([P, 256], F32, tag="pv", name="psv")
                for d in range(8):
                    nc.tensor.matmul(
                        ps[:],
                        lhsT=xt_sb[d][:, sc * P:(sc + 1) * P],
                        rhs=wv_sb[:, d * 256:(d + 1) * 256],
                        start=(d == 0),
                        stop=(d == 7),
                    )
                nc.gpsimd.memset(v_sb[sc][:].bitcast(F32), 1.0)
                nc.scalar.copy(
                    v_sb[sc][:].rearrange("p (h c) -> p h c", c=65)[:, :, 0:64],
                    ps[:].rearrange("p (h c) -> p h c", c=64),
                )

        # ---------------- Phase 2: attention ----------------
        with (
            tc.tile_pool(name="u", bufs=2) as u_pool,
            tc.tile_pool(name="psS", bufs=2, space="PSUM") as psS_pool,
            tc.tile_pool(name="psO", bufs=2, space="PSUM") as psO_pool,
        ):
            for h in range(NHC):
                m, p0 = h // 2, (h % 2) * DK
                for qs in range(4):
                    qw = slice(qs * 512, (qs + 1) * 512)
                    u_t = u_pool.tile([P, 16 * 512], F32R, tag="u", name="u_t")
                    psO = psO_pool.tile([P, 512], F32, tag="o", name="pso")
                    for kcp in range(8):
                        psS = psS_pool.tile([P, 1024], F32, tag="s", name="pss")
                        for half in range(2):
                            kc = 2 * kcp + half
                            nc.tensor.matmul(
                                psS[:, half * 512:(half + 1) * 512],
                                lhsT=kt_sb[m][p0:p0 + DK, kc * P:(kc + 1) * P],
                                rhs=qt_sb[m][p0:p0 + DK, qw],
                                start=True,
                                stop=True,
                            )
                        if "exp" not in _ABLATE:
                            nc.scalar.activation(
                                u_t[:, kcp * 1024:(kcp + 1) * 1024], psS[:], EXP, scale=SCALE
                            )
                        for half in range(2):
                            if "uv" in _ABLATE:
                                break
                            kc = 2 * kcp + half
                            nc.tensor.matmul(
                                psO[0:65, :],
                                lhsT=v_sb[kc][:, h * 65:(h + 1) * 65],
                                rhs=u_t[:, kc * 512:(kc + 1) * 512],
                                start=(kc == 0),
                                stop=(kc == 15),
                            )
                    rinv_t = rinv_pool.tile([1, 512], F32, tag="ri", name="rinv")
                    nc.vector.reciprocal(rinv_t[:], psO[64:65, :])
                    rb_t = rb_pool.tile([P, 512], F32, tag="rb", name="rb")
                    nc.gpsimd.partition_broadcast(rb_t[:], rinv_t[:])
                    # normalize U in-place; 256-wide ops stay under the DVE
                    # drain threshold (~266 ns)
                    if "norm" not in _ABLATE:
                        for kc in range(16):
                            for hf in range(2):
                                cw = slice(kc * 512 + hf * 256, kc * 512 + (hf + 1) * 256)
                                nc.vector.tensor_mul(
                                    u_t[:, cw], u_t[:, cw], rb_t[:, hf * 256:(hf + 1) * 256]
                                )
                    nc.vector.tensor_mul(
                        ao_sb[m][p0:p0 + DK, qw], psO[0:DK, :], rb_t[0:DK, :]
                    )
                    if "attn_dma" not in _ABLATE:
                        nc.sync.dma_start(
                            attnT_d[h].rearrange("(kc p) q -> p kc q", p=P)[:, :, qw],
                            u_t[:].rearrange("p (kc q) -> p kc q", q=512),
                        )

        # ---------------- Phase 3: output projection ----------------
        with (
            tc.tile_pool(name="psF", bufs=2, space="PSUM") as psF_pool,
            tc.tile_pool(name="oev", bufs=2) as oev_pool,
        ):
            for sc in range(16):
                psF = psF_pool.tile([P, D], F32, tag="f", name="psf")
                for n2 in range(2):
                    for t in range(2):
                        nc.tensor.matmul(
                            psF[:, n2 * 512:(n2 + 1) * 512],
                            lhsT=ao_sb[t][:, sc * P:(sc + 1) * P],
                            rhs=wo_sb[t][:, n2 * 512:(n2 + 1) * 512],
                            start=(t == 0),
                            stop=(t == 1),
                        )
                ot = oev_pool.tile([P, D], F32, tag="oe", name="oev")
                nc.scalar.copy(ot[:], psF[:])
                nc.sync.dma_start(outp_d[sc * P:(sc + 1) * P, :], ot[:])


def get_module():
    if "nc" not in _CACHE:
        _CACHE["nc"] = _build_module()
    return _CACHE["nc"]


def make_in_maps(x, Wq, bq, Wk, bk, Wv, Wo):
    """Shard the full inputs into the 8 per-core input maps."""
    x = np.ascontiguousarray(np.asarray(x, dtype=np.float32))
    xT = [np.ascontiguousarray(x[b].T) for b in range(x.shape[0])]
    in_maps = []
    for c in range(8):
        b, g = c // 4, c % 4
        cs = slice(g * 256, (g + 1) * 256)
        bq_g = np.asarray(bq)[cs]
        bk_g = np.asarray(bk)[cs]
        bqk = np.stack(
            [bq_g[0:128], bq_g[128:256], bk_g[0:128], bk_g[128:256]], axis=1
        ).astype(np.float32)
        in_maps.append(
            {
                "xT": xT[b],
                "wq": np.ascontiguousarray(np.asarray(Wq)[:, cs], dtype=np.float32),
                "wk": np.ascontiguousarray(np.asarray(Wk)[:, cs], dtype=np.float32),
                "wv": np.ascontiguousarray(np.asarray(Wv)[:, cs], dtype=np.float32),
                "wo": np.ascontiguousarray(np.asarray(Wo)[cs, :], dtype=np.float32),
                "bqk": np.ascontiguousarray(bqk),
            }
        )
    return in_maps


def assemble(results, Wv_bias_corr, B=2):
    """Gather the 8 per-core result dicts into full (out, attn_weights)."""
    out = np.empty((B, S, D), np.float32)
    attn = np.empty((B, 16, S, S), np.float32)
    for b in range(B):
        acc = None
        for g in range(4):
            r = results[b * 4 + g]
            acc = r["outp"].copy() if acc is None else acc + r["outp"]
            at = r["attnT"]
            for j in range(4):
                attn[b, 4 * g + j] = at[j].T
        out[b] = acc + Wv_bias_corr[None, :]
    return out, attn


def kernel(x, Wq, bq, Wk, bk, Wv, bv, Wo, bo):
    nc = get_module()
    in_maps = make_in_maps(x, Wq, bq, Wk, bk, Wv, Wo)
    res = bass_utils.run_bass_kernel_spmd(nc, in_maps, core_ids=list(range(8)))
    corr = (np.asarray(bv, np.float32) @ np.asarray(Wo, np.float32)
            + np.asarray(bo, np.float32))
    return assemble(res.results, corr)
